# revision 53
# baseline (speedup 1.0000x reference)
"""Trainium2 Bass kernel for nn_Bert (VOCAB=9, D=4, S=16384) on 8 NeuronCores.

Key identity: with a tiny vocabulary (9) and tiny width (4), every row of the
reference output depends only on the token id x[s] and the *global* histogram
c_v of x:

    T = emb @ proj_w.T + proj_b                       (9,4)  per-token h1
    G = T @ T.T                                       (9,9)  symmetric score table
    attn_out(a) = sum_v c_v e^{G[a,v]} T[v] / sum_v c_v e^{G[a,v]}
    F = softmax(relu(attn_out) @ M2.T + b2)           (9,9)  final per-token table
        where M2 = prj_w @ forw_w, b2 = prj_w @ forw_b + prj_b
        (the two affine layers after the relu compose into one)
    out[s] = F[x[s]]

Device schedule per core (sequence row-sharded, 2048 positions/core) — fully
hand-scheduled, no TileContext (its entry/exit all-engine barriers cost over
1us on a kernel this small). Cross-engine deps are explicit counting
semaphores; same-engine deps ride on queue order (accumulator-path producers
— tensor_scalar accum_out, tensor_reduce, reciprocal — additionally need a
semaphore even for same-engine consumers):

  - input DMAs issue immediately after the Bass-init barrier on SP/ACT/POOL
  - histogram of the full x: 8 fused is_equal+accum DVE ops (v=1..8) plus
    v=0 on the otherwise-idle ACT engine as relu(1 - x^2) with fused accum,
    all into one bf16 H tile (counts <= 128 are bf16-exact), so the c
    reduction is a single-pump bf16 matmul
  - 9x9 table math with the augmented [T | 1] operand folding relu/bias; Z
    rides in row 4 of the ShT matmul; T1/W/RTa/D2 are bf16 so the ShT/Z/P
    matmuls are single-pump instead of fp32 LOW/HIGH pairs (validated on the
    real inputs: rel err 5.5e-3 vs the 2e-2 gate); softmax tail
    exp(ACT) -> sum/recip/scale-to-bf16 (DVE)
  - F is bf16-only: the gather output is then exactly bf16(F[x_s]); no hi/lo
    split, 4 gather matmuls instead of 8
  - gather: F padded to [9, 32] so the four concurrent 512-column strips
    (tile_position col-tiling) write all 128 partitions of ONE psum bank
  - eviction: one ACT copy [128, 512] f32->bf16 (an ACT+DVE split pair hits
    the cayman cross-engine event-accel deadlock on HW, which is why Tile
    serializes such pairs)
  - single output DMA on SP gated only by the evict semaphore. No engine
    waits on its completion: the NEFF epilogue's per-engine semaphore-clear
    phase (~6us, which starts at the post-kernel rendezvous regardless)
    strictly covers the remaining DMA flight time (<2us), so the output
    always lands well before the NEFF can signal completion — and the
    epilogue overlaps the DMA tail instead of serializing after it
"""

import os

import ml_dtypes
import numpy as np

from concourse import bacc, mybir
from concourse._compat import get_trn_type
from concourse.bass_utils import run_bass_kernel_spmd

VOCAB = 9
D = 4
S = 16384
NCORES = 8
SLICE = S // NCORES  # 2048
NCHUNK = 4           # 512-column matmul chunks of the per-core slice
CHUNK = SLICE // NCHUNK

F32 = mybir.dt.float32
BF16 = mybir.dt.bfloat16

# Packed constants layout, one [128, 33] f32 tensor:
#   col 0      : ones (rows 0..127)
#   cols 1:5   : A  = [proj_w.T; proj_b]  rows 0..4   (K=5 augmented proj)
#   cols 5:14  : B  = [emb.T; ones(9)]   rows 0..4
#   cols 14:23 : D2 = [M2.T; b2]         rows 0..4    (folded forw+classifier)
#   col 23     : iota9 (rows 0..8 = 0..8)
#   cols 24:33 : spare
NCONST = 33

LAST_RESULTS = None  # BassKernelResults of the most recent run (for test.py)


def build_nc():
    nc = bacc.Bacc(
        get_trn_type() or "TRN2",
        target_bir_lowering=False,
        debug=False,
        enable_asserts=False,
        num_devices=NCORES,
    )
    xall = nc.dram_tensor("xall", [128, 128], BF16, kind="ExternalInput")
    xqrep = nc.dram_tensor("xqrep", [VOCAB, SLICE], BF16, kind="ExternalInput")
    consts = nc.dram_tensor("consts", [128, NCONST], F32, kind="ExternalInput")
    outT = nc.dram_tensor("outT", [128, CHUNK], BF16, kind="ExternalOutput")

    _build_kernel(nc, xall.ap(), xqrep.ap(), consts.ap(), outT.ap())
    nc.compile()
    return nc


def _build_kernel(nc, xall, xqrep, consts, outT):
    # counting semaphores: one per producing engine + DMA completions
    sPE = nc.alloc_semaphore("sPE")
    sDVE = nc.alloc_semaphore("sDVE")
    sACT = nc.alloc_semaphore("sACT")
    sPL = nc.alloc_semaphore("sPL")
    sA = nc.alloc_semaphore("sA")    # xall
    sC = nc.alloc_semaphore("sC")    # consts
    sQ = nc.alloc_semaphore("sQ")    # xqrep
    sO = nc.alloc_semaphore("sO")    # output

    # ---- PSUM: output bank first (full-bank [128, 512]); the tiny table
    # tensors share one bank at disjoint column ranges
    o_ps = nc.alloc_psum_tensor("o_ps", [128, CHUNK], F32).ap()
    small = nc.alloc_psum_tensor("small_ps", [128, 64], F32).ap()
    TT_ps = small[0:D, 0:VOCAB]
    T_ps = small[0:VOCAB, 9:13]
    G_ps = small[0:VOCAB, 13:22]
    c_ps = small[0:VOCAB, 22:23]
    ShTa_ps = small[0 : D + 1, 23:32]
    Z_ps = small[0:VOCAB, 32:33]
    P_ps = small[0:VOCAB, 33:42]

    # ---- SBUF
    x_s = nc.alloc_sbuf_tensor("x_s", [128, 128], BF16).ap()
    const_s = nc.alloc_sbuf_tensor("const_s", [128, NCONST], F32).ap()
    xq_s = nc.alloc_sbuf_tensor("xq_s", [VOCAB, SLICE], BF16).ap()
    ohb = nc.alloc_sbuf_tensor("ohb", [128, VOCAB, 128], BF16).ap()
    H = nc.alloc_sbuf_tensor("H", [128, VOCAB], BF16).ap()
    TT_s = nc.alloc_sbuf_tensor("TT_s", [D, VOCAB], F32).ap()
    T1_s = nc.alloc_sbuf_tensor("T1_s", [VOCAB, D + 1], BF16).ap()
    E_s = nc.alloc_sbuf_tensor("E_s", [VOCAB, VOCAB], F32).ap()
    W_s = nc.alloc_sbuf_tensor("W_s", [VOCAB, VOCAB], BF16).ap()
    RTa_s = nc.alloc_sbuf_tensor("RTa_s", [D + 1, VOCAB], BF16).ap()
    D2b_s = nc.alloc_sbuf_tensor("D2b_s", [5, VOCAB], BF16).ap()
    Zr_s = nc.alloc_sbuf_tensor("Zr_s", [VOCAB, 1], F32).ap()
    Sr_s = nc.alloc_sbuf_tensor("Sr_s", [VOCAB, 1], F32).ap()
    expL_s = nc.alloc_sbuf_tensor("expL_s", [VOCAB, VOCAB], F32).ap()
    Ssum_s = nc.alloc_sbuf_tensor("Ssum_s", [VOCAB, 1], F32).ap()
    Fhi_s = nc.alloc_sbuf_tensor("Fhi_s", [VOCAB, 32], BF16).ap()
    oh_s = nc.alloc_sbuf_tensor("oh_s", [VOCAB, SLICE], BF16).ap()
    outT_s = nc.alloc_sbuf_tensor("outT_s", [128, CHUNK], BF16).ap()
    sq_s = nc.alloc_sbuf_tensor("sq_s", [128, 128], BF16).ap()

    ones128 = const_s[0:128, 0:1]
    ones9 = const_s[0:VOCAB, 0:1]
    A_s = const_s[0:5, 1:5]
    B_s = const_s[0:5, 5:14]
    D2_s = const_s[0:5, 14:23]
    iota9 = const_s[0:VOCAB, 23:24]
    ones128_bf = nc.const_aps.aps[(BF16, 1.0)]
    ones9_bf = ones128_bf[0:VOCAB, 0:1]

    # ================= SP: input DMA, then the gated output DMA =============
    nc.sync.dma_start(x_s, xall).then_inc(sA, 16)

    # ================= ACT: consts DMA + table copies + activations =========
    nc.scalar.dma_start(const_s, consts).then_inc(sC, 16)

    # ================= POOL: xq DMA (SWDGE) + constant memsets ==============
    nc.gpsimd.dma_start(xq_s, xqrep).then_inc(sQ, 16)
    nc.gpsimd.memset(T1_s, 1.0).then_inc(sPL, 1)
    nc.gpsimd.memset(Fhi_s, 0.0).then_inc(sPL, 1)

    # ================= DVE: histogram (9 fused is_equal+accum ops) ==========
    nc.vector.wait_ge(sA, 16)
    for v in range(1, VOCAB):
        nc.vector.tensor_scalar(
            out=ohb[:, v, :],
            in0=x_s,
            scalar1=float(v),
            scalar2=None,
            op0=mybir.AluOpType.is_equal,
            op1=mybir.AluOpType.add,
            accum_out=H[:, v : v + 1],
        ).then_inc(sDVE, 1)

    # ================= ACT: histogram value v=0 while waiting for consts ====
    # relu(1 - x^2) = [x == 0] exactly for integer tokens; the fused accum
    # gives the column sum, taking one op off the DVE histogram's 9
    nc.scalar.wait_ge(sA, 16)
    nc.scalar.activation(
        sq_s, x_s, mybir.ActivationFunctionType.Square
    ).then_inc(sACT, 1)
    nc.scalar.wait_ge(sACT, 1)
    with nc.allow_low_precision(reason="counts <= 128 are exact in bf16"):
        nc.scalar.activation(
            ohb[:, 0, :], sq_s, mybir.ActivationFunctionType.Relu,
            bias=1.0, scale=-1.0, accum_out=H[:, 0:1],
        ).then_inc(sACT, 1)

    # ================= PE: tables (queue order keeps them before c) =========
    nc.tensor.wait_ge(sC, 16)
    nc.tensor.matmul(TT_ps, A_s, B_s).then_inc(sPE, 1)
    nc.tensor.matmul(T_ps, B_s, A_s).then_inc(sPE, 1)

    # ACT: TT_s / T1 copies + E
    nc.scalar.wait_ge(sPE, 1)
    nc.scalar.copy(TT_s, TT_ps).then_inc(sACT, 1)
    nc.scalar.wait_ge(sPE, 2)
    nc.scalar.wait_ge(sPL, 1)
    nc.scalar.copy(T1_s[:, 0:D], T_ps).then_inc(sACT, 1)
    nc.scalar.copy(D2b_s, D2_s).then_inc(sACT, 1)

    nc.tensor.wait_ge(sACT, 3)
    nc.tensor.matmul(G_ps, TT_s, TT_s).then_inc(sPE, 1)
    nc.scalar.wait_ge(sPE, 3)
    nc.scalar.activation(
        E_s, G_ps, mybir.ActivationFunctionType.Exp
    ).then_inc(sACT, 1)

    # c[v] = sum_p H[p, v] — bf16 operands are exact counts, single-pump mm
    nc.tensor.wait_ge(sDVE, 8)
    nc.tensor.wait_ge(sACT, 2)
    nc.tensor.matmul(c_ps, H, ones128_bf).then_inc(sPE, 1)

    # W[v, a] = c_v * exp(G[v, a])
    nc.vector.wait_ge(sPE, 4)
    nc.vector.wait_ge(sACT, 6)
    nc.vector.tensor_scalar(
        out=W_s, in0=E_s, scalar1=c_ps, scalar2=None, op0=mybir.AluOpType.mult
    ).then_inc(sDVE, 1)

    # rows 0-3 = Sh^T, row 4 = Z; Z column for the per-partition exp scale
    nc.tensor.wait_ge(sDVE, 9)
    nc.tensor.matmul(ShTa_ps, T1_s, W_s).then_inc(sPE, 1)
    nc.tensor.matmul(Z_ps, W_s, ones9_bf).then_inc(sPE, 1)

    nc.scalar.wait_ge(sPE, 5)
    nc.scalar.activation(
        RTa_s, ShTa_ps, mybir.ActivationFunctionType.Relu
    ).then_inc(sACT, 1)

    # one-hot (after W in DVE queue order), then Zr
    nc.vector.wait_ge(sQ, 16)
    nc.vector.tensor_scalar(
        out=oh_s,
        in0=xq_s,
        scalar1=iota9,
        scalar2=None,
        op0=mybir.AluOpType.is_equal,
    ).then_inc(sDVE, 1)
    nc.vector.wait_ge(sPE, 6)
    nc.vector.reciprocal(Zr_s, Z_ps).then_inc(sDVE, 1)

    nc.tensor.wait_ge(sACT, 7)
    nc.tensor.matmul(P_ps, RTa_s, D2b_s).then_inc(sPE, 1)

    # softmax tail: exp on ACT, sum/recip/scale on DVE
    nc.scalar.wait_ge(sPE, 7)
    nc.scalar.wait_ge(sDVE, 11)
    nc.scalar.activation(
        expL_s, P_ps, mybir.ActivationFunctionType.Exp, scale=Zr_s
    ).then_inc(sACT, 1)
    nc.vector.wait_ge(sACT, 8)
    nc.vector.tensor_reduce(
        Ssum_s, expL_s, axis=mybir.AxisListType.X, op=mybir.AluOpType.add
    ).then_inc(sDVE, 1)
    # the reduce writes via the accumulator path: even same-engine consumers
    # need a semaphore on its completion
    nc.vector.wait_ge(sDVE, 12)
    nc.vector.reciprocal(Sr_s, Ssum_s).then_inc(sDVE, 1)
    nc.vector.wait_ge(sDVE, 13)
    nc.vector.wait_ge(sPL, 2)
    nc.vector.tensor_scalar(
        out=Fhi_s[:, 0:VOCAB],
        in0=expL_s,
        scalar1=Sr_s,
        scalar2=None,
        op0=mybir.AluOpType.mult,
    ).then_inc(sDVE, 1)

    # gather: four concurrent 32-col strips into one psum bank
    nc.tensor.wait_ge(sDVE, 14)
    for cidx in range(NCHUNK):
        sl = slice(cidx * CHUNK, (cidx + 1) * CHUNK)
        nc.tensor.matmul(
            o_ps[32 * cidx : 32 * cidx + 32, :],
            Fhi_s,
            oh_s[:, sl],
            start=True,
            stop=True,
            tile_position=(0, 32 * cidx),
            skip_group_check=True,
        ).then_inc(sPE, 1)

    # eviction: one ACT copy f32 -> bf16 (ACT+DVE split halves hit the
    # cayman event-accel cross-engine deadlock on HW — tile serializes the
    # pair for the same reason — so a single copy is both safe and as fast)
    nc.scalar.wait_ge(sPE, 11)
    nc.scalar.copy(outT_s, o_ps).then_inc(sACT, 1)

    # output DMA on SP, gated on the evict. No engine waits on the completion
    # semaphore: the NEFF epilogue's per-engine semaphore-clear phase (~6us,
    # started at the post-kernel rendezvous) strictly covers the remaining
    # DMA flight time, so the output always lands well before the NEFF can
    # signal completion.
    nc.sync.wait_ge(sACT, 9)
    nc.sync.dma_start(outT, outT_s).then_inc(sO, 16)


def host_prep(x, emb, proj_w, proj_b, forw_w, forw_b, prj_w, prj_b):
    """Pack weights/constants and per-core sharded inputs."""
    f32 = np.float32
    x = np.asarray(x).reshape(-1).astype(np.int64)
    assert x.shape == (S,)
    emb = np.asarray(emb, f32)
    proj_w = np.asarray(proj_w, f32)
    proj_b = np.asarray(proj_b, f32)
    forw_w = np.asarray(forw_w, f32)
    forw_b = np.asarray(forw_b, f32)
    prj_w = np.asarray(prj_w, f32)
    prj_b = np.asarray(prj_b, f32)

    M2 = (prj_w @ forw_w).astype(f32)          # (9, 4)
    b2 = (prj_w @ forw_b + prj_b).astype(f32)  # (9,)

    consts = np.zeros((128, NCONST), f32)
    consts[:, 0] = 1.0
    consts[0:4, 1:5] = proj_w.T
    consts[4, 1:5] = proj_b
    consts[0:4, 5:14] = emb.T
    consts[4, 5:14] = 1.0
    consts[0:4, 14:23] = M2.T
    consts[4, 14:23] = b2
    consts[0:VOCAB, 23] = np.arange(VOCAB, dtype=f32)

    xall = x.reshape(128, 128).astype(ml_dtypes.bfloat16)
    in_maps = []
    for i in range(NCORES):
        xq = x[i * SLICE : (i + 1) * SLICE].astype(ml_dtypes.bfloat16)
        in_maps.append(
            {
                "xall": xall,
                "consts": consts,
                "xqrep": np.ascontiguousarray(
                    np.broadcast_to(xq[None, :], (VOCAB, SLICE))
                ),
            }
        )
    return in_maps


def unpack_out(arr):
    """outT [128, CHUNK] bf16 -> (SLICE, VOCAB) f32 for one core."""
    a = np.asarray(arr).astype(np.float32)
    return a.reshape(NCHUNK, 32, CHUNK)[:, :VOCAB, :].transpose(0, 2, 1).reshape(
        SLICE, VOCAB
    )


_NC_CACHE = None


def kernel(x, emb, proj_w, proj_b, forw_w, forw_b, prj_w, prj_b):
    global _NC_CACHE, LAST_RESULTS
    if _NC_CACHE is None:
        _NC_CACHE = build_nc()
    nc = _NC_CACHE
    in_maps = host_prep(x, emb, proj_w, proj_b, forw_w, forw_b, prj_w, prj_b)
    trace = bool(os.environ.get("BASS_TRACE"))
    res = run_bass_kernel_spmd(nc, in_maps, list(range(NCORES)), trace=trace)
    LAST_RESULTS = res
    out = np.empty((S, VOCAB), np.float32)
    for i in range(NCORES):
        out[i * SLICE : (i + 1) * SLICE, :] = unpack_out(res.results[i]["outT"])
    return out


# revision 54
# speedup vs baseline: 1.0075x; 1.0075x over previous
"""Trainium2 Bass kernel for nn_Bert (VOCAB=9, D=4, S=16384) on 8 NeuronCores.

Key identity: with a tiny vocabulary (9) and tiny width (4), every row of the
reference output depends only on the token id x[s] and the *global* histogram
c_v of x:

    T = emb @ proj_w.T + proj_b                       (9,4)  per-token h1
    G = T @ T.T                                       (9,9)  symmetric score table
    attn_out(a) = sum_v c_v e^{G[a,v]} T[v] / sum_v c_v e^{G[a,v]}
    F = softmax(relu(attn_out) @ M2.T + b2)           (9,9)  final per-token table
        where M2 = prj_w @ forw_w, b2 = prj_w @ forw_b + prj_b
        (the two affine layers after the relu compose into one)
    out[s] = F[x[s]]

Device schedule per core (sequence row-sharded, 2048 positions/core) — fully
hand-scheduled, no TileContext (its entry/exit all-engine barriers cost over
1us on a kernel this small). Cross-engine deps are explicit counting
semaphores; same-engine deps ride on queue order (accumulator-path producers
— tensor_scalar accum_out, tensor_reduce, reciprocal — additionally need a
semaphore even for same-engine consumers):

  - input DMAs issue immediately after the Bass-init barrier on SP/ACT/POOL
  - histogram of the full x: 8 fused is_equal+accum DVE ops (v=1..8) plus
    v=0 on the otherwise-idle ACT engine as relu(1 - x^2) with fused accum,
    all into one bf16 H tile (counts <= 128 are bf16-exact), so the c
    reduction is a single-pump bf16 matmul
  - 9x9 table math with the augmented [T | 1] operand folding relu/bias; Z
    rides in row 4 of the ShT matmul; T1/W/RTa/D2 are bf16 so the ShT/Z/P
    matmuls are single-pump instead of fp32 LOW/HIGH pairs (validated on the
    real inputs: rel err 5.5e-3 vs the 2e-2 gate); softmax tail
    exp(ACT) -> sum/recip/scale-to-bf16 (DVE)
  - F is bf16-only: the gather output is then exactly bf16(F[x_s]); no hi/lo
    split, 4 gather matmuls instead of 8
  - gather: F padded to [9, 32] so the four concurrent 512-column strips
    (tile_position col-tiling) write all 128 partitions of ONE psum bank
  - eviction: one ACT copy [128, 512] f32->bf16 (an ACT+DVE split pair hits
    the cayman cross-engine event-accel deadlock on HW, which is why Tile
    serializes such pairs)
  - single output DMA on SP gated only by the evict semaphore. No engine
    waits on its completion: the NEFF epilogue's per-engine semaphore-clear
    phase (~6us, which starts at the post-kernel rendezvous regardless)
    strictly covers the remaining DMA flight time (<2us), so the output
    always lands well before the NEFF can signal completion — and the
    epilogue overlaps the DMA tail instead of serializing after it
"""

import os

import ml_dtypes
import numpy as np

from concourse import bacc, mybir
from concourse._compat import get_trn_type
from concourse.bass_utils import run_bass_kernel_spmd

VOCAB = 9
D = 4
S = 16384
NCORES = 8
SLICE = S // NCORES  # 2048
NCHUNK = 4           # 512-column matmul chunks of the per-core slice
CHUNK = SLICE // NCHUNK

F32 = mybir.dt.float32
BF16 = mybir.dt.bfloat16

# Packed constants layout, one [128, 33] f32 tensor:
#   col 0      : ones (rows 0..127)
#   cols 1:5   : A  = [proj_w.T; proj_b]  rows 0..4   (K=5 augmented proj)
#   cols 5:14  : B  = [emb.T; ones(9)]   rows 0..4
#   cols 14:23 : D2 = [M2.T; b2]         rows 0..4    (folded forw+classifier)
#   col 23     : iota9 (rows 0..8 = 0..8)
#   cols 24:33 : spare
NCONST = 33

LAST_RESULTS = None  # BassKernelResults of the most recent run (for test.py)


def build_nc():
    nc = bacc.Bacc(
        get_trn_type() or "TRN2",
        target_bir_lowering=False,
        debug=False,
        enable_asserts=False,
        num_devices=NCORES,
    )
    xall = nc.dram_tensor("xall", [128, 128], mybir.dt.uint8, kind="ExternalInput")
    xqrep = nc.dram_tensor("xqrep", [VOCAB, SLICE], BF16, kind="ExternalInput")
    consts = nc.dram_tensor("consts", [128, NCONST], F32, kind="ExternalInput")
    outT = nc.dram_tensor("outT", [128, CHUNK], BF16, kind="ExternalOutput")

    _build_kernel(nc, xall.ap(), xqrep.ap(), consts.ap(), outT.ap())
    nc.compile()
    return nc


def _build_kernel(nc, xall, xqrep, consts, outT):
    # counting semaphores: one per producing engine + DMA completions
    sPE = nc.alloc_semaphore("sPE")
    sDVE = nc.alloc_semaphore("sDVE")
    sACT = nc.alloc_semaphore("sACT")
    sPL = nc.alloc_semaphore("sPL")
    sA = nc.alloc_semaphore("sA")    # xall
    sC = nc.alloc_semaphore("sC")    # consts
    sQ = nc.alloc_semaphore("sQ")    # xqrep
    sO = nc.alloc_semaphore("sO")    # output

    # ---- PSUM: output bank first (full-bank [128, 512]); the tiny table
    # tensors share one bank at disjoint column ranges
    o_ps = nc.alloc_psum_tensor("o_ps", [128, CHUNK], F32).ap()
    small = nc.alloc_psum_tensor("small_ps", [128, 64], F32).ap()
    TT_ps = small[0:D, 0:VOCAB]
    T_ps = small[0:VOCAB, 9:13]
    G_ps = small[0:VOCAB, 13:22]
    c_ps = small[0:VOCAB, 22:23]
    ShTa_ps = small[0 : D + 1, 23:32]
    Z_ps = small[0:VOCAB, 32:33]
    P_ps = small[0:VOCAB, 33:42]

    # ---- SBUF
    x_s = nc.alloc_sbuf_tensor("x_s", [128, 128], mybir.dt.uint8).ap()
    const_s = nc.alloc_sbuf_tensor("const_s", [128, NCONST], F32).ap()
    xq_s = nc.alloc_sbuf_tensor("xq_s", [VOCAB, SLICE], BF16).ap()
    ohb = nc.alloc_sbuf_tensor("ohb", [128, VOCAB, 128], BF16).ap()
    H = nc.alloc_sbuf_tensor("H", [128, VOCAB], BF16).ap()
    TT_s = nc.alloc_sbuf_tensor("TT_s", [D, VOCAB], F32).ap()
    T1_s = nc.alloc_sbuf_tensor("T1_s", [VOCAB, D + 1], BF16).ap()
    E_s = nc.alloc_sbuf_tensor("E_s", [VOCAB, VOCAB], F32).ap()
    W_s = nc.alloc_sbuf_tensor("W_s", [VOCAB, VOCAB], BF16).ap()
    RTa_s = nc.alloc_sbuf_tensor("RTa_s", [D + 1, VOCAB], BF16).ap()
    D2b_s = nc.alloc_sbuf_tensor("D2b_s", [5, VOCAB], BF16).ap()
    Zr_s = nc.alloc_sbuf_tensor("Zr_s", [VOCAB, 1], F32).ap()
    Sr_s = nc.alloc_sbuf_tensor("Sr_s", [VOCAB, 1], F32).ap()
    expL_s = nc.alloc_sbuf_tensor("expL_s", [VOCAB, VOCAB], F32).ap()
    Ssum_s = nc.alloc_sbuf_tensor("Ssum_s", [VOCAB, 1], F32).ap()
    Fhi_s = nc.alloc_sbuf_tensor("Fhi_s", [VOCAB, 32], BF16).ap()
    oh_s = nc.alloc_sbuf_tensor("oh_s", [VOCAB, SLICE], BF16).ap()
    outT_s = nc.alloc_sbuf_tensor("outT_s", [128, CHUNK], BF16).ap()
    sq_s = nc.alloc_sbuf_tensor("sq_s", [128, 128], BF16).ap()

    ones128 = const_s[0:128, 0:1]
    ones9 = const_s[0:VOCAB, 0:1]
    A_s = const_s[0:5, 1:5]
    B_s = const_s[0:5, 5:14]
    D2_s = const_s[0:5, 14:23]
    iota9 = const_s[0:VOCAB, 23:24]
    ones128_bf = nc.const_aps.aps[(BF16, 1.0)]
    ones9_bf = ones128_bf[0:VOCAB, 0:1]

    # ================= SP: input DMA, then the gated output DMA =============
    nc.sync.dma_start(x_s, xall).then_inc(sA, 16)

    # ================= ACT: consts DMA + table copies + activations =========
    nc.scalar.dma_start(const_s, consts).then_inc(sC, 16)

    # ================= POOL: xq DMA (SWDGE) + constant memsets ==============
    nc.gpsimd.dma_start(xq_s, xqrep).then_inc(sQ, 16)
    nc.gpsimd.memset(T1_s, 1.0).then_inc(sPL, 1)
    nc.gpsimd.memset(Fhi_s, 0.0).then_inc(sPL, 1)

    # ================= DVE: histogram (9 fused is_equal+accum ops) ==========
    nc.vector.wait_ge(sA, 16)
    for v in range(1, VOCAB):
        nc.vector.tensor_scalar(
            out=ohb[:, v, :],
            in0=x_s,
            scalar1=float(v),
            scalar2=None,
            op0=mybir.AluOpType.is_equal,
            op1=mybir.AluOpType.add,
            accum_out=H[:, v : v + 1],
        ).then_inc(sDVE, 1)

    # ================= ACT: histogram value v=0 while waiting for consts ====
    # relu(1 - x^2) = [x == 0] exactly for integer tokens; the fused accum
    # gives the column sum, taking one op off the DVE histogram's 9
    nc.scalar.wait_ge(sA, 16)
    nc.scalar.activation(
        sq_s, x_s, mybir.ActivationFunctionType.Square
    ).then_inc(sACT, 1)
    nc.scalar.wait_ge(sACT, 1)
    with nc.allow_low_precision(reason="counts <= 128 are exact in bf16"):
        nc.scalar.activation(
            ohb[:, 0, :], sq_s, mybir.ActivationFunctionType.Relu,
            bias=1.0, scale=-1.0, accum_out=H[:, 0:1],
        ).then_inc(sACT, 1)

    # ================= PE: tables (queue order keeps them before c) =========
    nc.tensor.wait_ge(sC, 16)
    nc.tensor.matmul(TT_ps, A_s, B_s).then_inc(sPE, 1)
    nc.tensor.matmul(T_ps, B_s, A_s).then_inc(sPE, 1)

    # ACT: TT_s / T1 copies + E
    nc.scalar.wait_ge(sPE, 1)
    nc.scalar.copy(TT_s, TT_ps).then_inc(sACT, 1)
    nc.scalar.wait_ge(sPE, 2)
    nc.scalar.wait_ge(sPL, 1)
    nc.scalar.copy(T1_s[:, 0:D], T_ps).then_inc(sACT, 1)
    nc.scalar.copy(D2b_s, D2_s).then_inc(sACT, 1)

    nc.tensor.wait_ge(sACT, 3)
    nc.tensor.matmul(G_ps, TT_s, TT_s).then_inc(sPE, 1)
    nc.scalar.wait_ge(sPE, 3)
    nc.scalar.activation(
        E_s, G_ps, mybir.ActivationFunctionType.Exp
    ).then_inc(sACT, 1)

    # c[v] = sum_p H[p, v] — bf16 operands are exact counts, single-pump mm
    nc.tensor.wait_ge(sDVE, 8)
    nc.tensor.wait_ge(sACT, 2)
    nc.tensor.matmul(c_ps, H, ones128_bf).then_inc(sPE, 1)

    # W[v, a] = c_v * exp(G[v, a])
    nc.vector.wait_ge(sPE, 4)
    nc.vector.wait_ge(sACT, 6)
    nc.vector.tensor_scalar(
        out=W_s, in0=E_s, scalar1=c_ps, scalar2=None, op0=mybir.AluOpType.mult
    ).then_inc(sDVE, 1)

    # rows 0-3 = Sh^T, row 4 = Z; Z column for the per-partition exp scale
    nc.tensor.wait_ge(sDVE, 9)
    nc.tensor.matmul(ShTa_ps, T1_s, W_s).then_inc(sPE, 1)
    nc.tensor.matmul(Z_ps, W_s, ones9_bf).then_inc(sPE, 1)

    nc.scalar.wait_ge(sPE, 5)
    nc.scalar.activation(
        RTa_s, ShTa_ps, mybir.ActivationFunctionType.Relu
    ).then_inc(sACT, 1)

    # one-hot (after W in DVE queue order), then Zr
    nc.vector.wait_ge(sQ, 16)
    nc.vector.tensor_scalar(
        out=oh_s,
        in0=xq_s,
        scalar1=iota9,
        scalar2=None,
        op0=mybir.AluOpType.is_equal,
    ).then_inc(sDVE, 1)
    nc.vector.wait_ge(sPE, 6)
    nc.vector.reciprocal(Zr_s, Z_ps).then_inc(sDVE, 1)

    nc.tensor.wait_ge(sACT, 7)
    nc.tensor.matmul(P_ps, RTa_s, D2b_s).then_inc(sPE, 1)

    # softmax tail: exp on ACT, sum/recip/scale on DVE
    nc.scalar.wait_ge(sPE, 7)
    nc.scalar.wait_ge(sDVE, 11)
    nc.scalar.activation(
        expL_s, P_ps, mybir.ActivationFunctionType.Exp, scale=Zr_s
    ).then_inc(sACT, 1)
    nc.vector.wait_ge(sACT, 8)
    nc.vector.tensor_reduce(
        Ssum_s, expL_s, axis=mybir.AxisListType.X, op=mybir.AluOpType.add
    ).then_inc(sDVE, 1)
    # the reduce writes via the accumulator path: even same-engine consumers
    # need a semaphore on its completion
    nc.vector.wait_ge(sDVE, 12)
    nc.vector.reciprocal(Sr_s, Ssum_s).then_inc(sDVE, 1)
    nc.vector.wait_ge(sDVE, 13)
    nc.vector.wait_ge(sPL, 2)
    nc.vector.tensor_scalar(
        out=Fhi_s[:, 0:VOCAB],
        in0=expL_s,
        scalar1=Sr_s,
        scalar2=None,
        op0=mybir.AluOpType.mult,
    ).then_inc(sDVE, 1)

    # gather: four concurrent 32-col strips into one psum bank
    nc.tensor.wait_ge(sDVE, 14)
    for cidx in range(NCHUNK):
        sl = slice(cidx * CHUNK, (cidx + 1) * CHUNK)
        nc.tensor.matmul(
            o_ps[32 * cidx : 32 * cidx + 32, :],
            Fhi_s,
            oh_s[:, sl],
            start=True,
            stop=True,
            tile_position=(0, 32 * cidx),
            skip_group_check=True,
        ).then_inc(sPE, 1)

    # eviction: one ACT copy f32 -> bf16 (ACT+DVE split halves hit the
    # cayman event-accel cross-engine deadlock on HW — tile serializes the
    # pair for the same reason — so a single copy is both safe and as fast)
    nc.scalar.wait_ge(sPE, 11)
    nc.scalar.copy(outT_s, o_ps).then_inc(sACT, 1)

    # output DMA on SP, gated on the evict. No engine waits on the completion
    # semaphore: the NEFF epilogue's per-engine semaphore-clear phase (~6us,
    # started at the post-kernel rendezvous) strictly covers the remaining
    # DMA flight time, so the output always lands well before the NEFF can
    # signal completion.
    nc.sync.wait_ge(sACT, 9)
    nc.sync.dma_start(outT, outT_s).then_inc(sO, 16)


def host_prep(x, emb, proj_w, proj_b, forw_w, forw_b, prj_w, prj_b):
    """Pack weights/constants and per-core sharded inputs."""
    f32 = np.float32
    x = np.asarray(x).reshape(-1).astype(np.int64)
    assert x.shape == (S,)
    emb = np.asarray(emb, f32)
    proj_w = np.asarray(proj_w, f32)
    proj_b = np.asarray(proj_b, f32)
    forw_w = np.asarray(forw_w, f32)
    forw_b = np.asarray(forw_b, f32)
    prj_w = np.asarray(prj_w, f32)
    prj_b = np.asarray(prj_b, f32)

    M2 = (prj_w @ forw_w).astype(f32)          # (9, 4)
    b2 = (prj_w @ forw_b + prj_b).astype(f32)  # (9,)

    consts = np.zeros((128, NCONST), f32)
    consts[:, 0] = 1.0
    consts[0:4, 1:5] = proj_w.T
    consts[4, 1:5] = proj_b
    consts[0:4, 5:14] = emb.T
    consts[4, 5:14] = 1.0
    consts[0:4, 14:23] = M2.T
    consts[4, 14:23] = b2
    consts[0:VOCAB, 23] = np.arange(VOCAB, dtype=f32)

    xall = x.reshape(128, 128).astype(np.uint8)
    in_maps = []
    for i in range(NCORES):
        xq = x[i * SLICE : (i + 1) * SLICE].astype(ml_dtypes.bfloat16)
        in_maps.append(
            {
                "xall": xall,
                "consts": consts,
                "xqrep": np.ascontiguousarray(
                    np.broadcast_to(xq[None, :], (VOCAB, SLICE))
                ),
            }
        )
    return in_maps


def unpack_out(arr):
    """outT [128, CHUNK] bf16 -> (SLICE, VOCAB) f32 for one core."""
    a = np.asarray(arr).astype(np.float32)
    return a.reshape(NCHUNK, 32, CHUNK)[:, :VOCAB, :].transpose(0, 2, 1).reshape(
        SLICE, VOCAB
    )


_NC_CACHE = None


def kernel(x, emb, proj_w, proj_b, forw_w, forw_b, prj_w, prj_b):
    global _NC_CACHE, LAST_RESULTS
    if _NC_CACHE is None:
        _NC_CACHE = build_nc()
    nc = _NC_CACHE
    in_maps = host_prep(x, emb, proj_w, proj_b, forw_w, forw_b, prj_w, prj_b)
    trace = bool(os.environ.get("BASS_TRACE"))
    res = run_bass_kernel_spmd(nc, in_maps, list(range(NCORES)), trace=trace)
    LAST_RESULTS = res
    out = np.empty((S, VOCAB), np.float32)
    for i in range(NCORES):
        out[i * SLICE : (i + 1) * SLICE, :] = unpack_out(res.results[i]["outT"])
    return out


# revision 55
# speedup vs baseline: 1.0135x; 1.0059x over previous
"""Trainium2 Bass kernel for nn_Bert (VOCAB=9, D=4, S=16384) on 8 NeuronCores.

Key identity: with a tiny vocabulary (9) and tiny width (4), every row of the
reference output depends only on the token id x[s] and the *global* histogram
c_v of x:

    T = emb @ proj_w.T + proj_b                       (9,4)  per-token h1
    G = T @ T.T                                       (9,9)  symmetric score table
    attn_out(a) = sum_v c_v e^{G[a,v]} T[v] / sum_v c_v e^{G[a,v]}
    F = softmax(relu(attn_out) @ M2.T + b2)           (9,9)  final per-token table
        where M2 = prj_w @ forw_w, b2 = prj_w @ forw_b + prj_b
        (the two affine layers after the relu compose into one)
    out[s] = F[x[s]]

Device schedule per core (sequence row-sharded, 2048 positions/core) — fully
hand-scheduled, no TileContext (its entry/exit all-engine barriers cost over
1us on a kernel this small). Cross-engine deps are explicit counting
semaphores; same-engine deps ride on queue order (accumulator-path producers
— tensor_scalar accum_out, tensor_reduce, reciprocal — additionally need a
semaphore even for same-engine consumers):

  - input DMAs issue immediately after the Bass-init barrier on SP/ACT/POOL
  - histogram of the full x: 8 fused is_equal+accum DVE ops (v=1..8) plus
    v=0 on the otherwise-idle ACT engine as relu(1 - x^2) with fused accum,
    all into one bf16 H tile (counts <= 128 are bf16-exact), so the c
    reduction is a single-pump bf16 matmul
  - 9x9 table math with the augmented [T | 1] operand folding relu/bias; Z
    rides in row 4 of the ShT matmul; T1/W/RTa/D2 are bf16 so the ShT/Z/P
    matmuls are single-pump instead of fp32 LOW/HIGH pairs (validated on the
    real inputs: rel err 5.5e-3 vs the 2e-2 gate); softmax tail
    exp(ACT) -> sum/recip/scale-to-bf16 (DVE)
  - F is bf16-only: the gather output is then exactly bf16(F[x_s]); no hi/lo
    split, 4 gather matmuls instead of 8
  - gather: F padded to [9, 32] so the four concurrent 512-column strips
    (tile_position col-tiling) write all 128 partitions of ONE psum bank
  - eviction: one ACT copy [128, 512] f32->bf16 (an ACT+DVE split pair hits
    the cayman cross-engine event-accel deadlock on HW, which is why Tile
    serializes such pairs)
  - single output DMA on SP gated only by the evict semaphore. No engine
    waits on its completion: the NEFF epilogue's per-engine semaphore-clear
    phase (~6us, which starts at the post-kernel rendezvous regardless)
    strictly covers the remaining DMA flight time (<2us), so the output
    always lands well before the NEFF can signal completion — and the
    epilogue overlaps the DMA tail instead of serializing after it
"""

import os

import ml_dtypes
import numpy as np

from concourse import bacc, mybir
from concourse._compat import get_trn_type
from concourse.bass_utils import run_bass_kernel_spmd

VOCAB = 9
D = 4
S = 16384
NCORES = 8
SLICE = S // NCORES  # 2048
NCHUNK = 4           # 512-column matmul chunks of the per-core slice
CHUNK = SLICE // NCHUNK

F32 = mybir.dt.float32
BF16 = mybir.dt.bfloat16

# Packed constants layout, one [128, 33] f32 tensor:
#   col 0      : ones (rows 0..127)
#   cols 1:5   : A  = [proj_w.T; proj_b]  rows 0..4   (K=5 augmented proj)
#   cols 5:14  : B  = [emb.T; ones(9)]   rows 0..4
#   cols 14:23 : D2 = [M2.T; b2]         rows 0..4    (folded forw+classifier)
#   col 23     : iota9 (rows 0..8 = 0..8)
#   cols 24:33 : spare
NCONST = 33

LAST_RESULTS = None  # BassKernelResults of the most recent run (for test.py)


def build_nc():
    nc = bacc.Bacc(
        get_trn_type() or "TRN2",
        target_bir_lowering=False,
        debug=False,
        enable_asserts=False,
        num_devices=NCORES,
    )
    xall = nc.dram_tensor("xall", [128, 128], mybir.dt.uint8, kind="ExternalInput")
    xqrep = nc.dram_tensor("xqrep", [VOCAB, SLICE], BF16, kind="ExternalInput")
    consts = nc.dram_tensor("consts", [128, NCONST], F32, kind="ExternalInput")
    outT = nc.dram_tensor("outT", [128, CHUNK], BF16, kind="ExternalOutput")

    _build_kernel(nc, xall.ap(), xqrep.ap(), consts.ap(), outT.ap())
    nc.compile()
    return nc


def _build_kernel(nc, xall, xqrep, consts, outT):
    # counting semaphores: one per producing engine + DMA completions
    sPE = nc.alloc_semaphore("sPE")
    sDVE = nc.alloc_semaphore("sDVE")
    sACT = nc.alloc_semaphore("sACT")
    sPL = nc.alloc_semaphore("sPL")
    sA = nc.alloc_semaphore("sA")    # xall
    sC = nc.alloc_semaphore("sC")    # consts
    sQ = nc.alloc_semaphore("sQ")    # xqrep
    sO = nc.alloc_semaphore("sO")    # output

    # ---- PSUM: output bank first (full-bank [128, 512]); the tiny table
    # tensors share one bank at disjoint column ranges
    o_ps = nc.alloc_psum_tensor("o_ps", [128, CHUNK], F32).ap()
    small = nc.alloc_psum_tensor("small_ps", [128, 64], F32).ap()
    TT_ps = small[0:D, 0:VOCAB]
    T_ps = small[0:VOCAB, 9:13]
    G_ps = small[0:VOCAB, 13:22]
    c_ps = small[0:VOCAB, 22:23]
    ShTa_ps = small[0 : D + 1, 23:32]
    Z_ps = small[0:VOCAB, 32:33]
    P_ps = small[0:VOCAB, 33:42]

    # ---- SBUF
    x_s = nc.alloc_sbuf_tensor("x_s", [128, 128], mybir.dt.uint8).ap()
    const_s = nc.alloc_sbuf_tensor("const_s", [128, NCONST], F32).ap()
    xq_s = nc.alloc_sbuf_tensor("xq_s", [VOCAB, SLICE], BF16).ap()
    ohb = nc.alloc_sbuf_tensor("ohb", [128, VOCAB, 128], BF16).ap()
    H = nc.alloc_sbuf_tensor("H", [128, VOCAB], BF16).ap()
    TT_s = nc.alloc_sbuf_tensor("TT_s", [D, VOCAB], F32).ap()
    T1_s = nc.alloc_sbuf_tensor("T1_s", [VOCAB, D + 1], BF16).ap()
    E_s = nc.alloc_sbuf_tensor("E_s", [VOCAB, VOCAB], F32).ap()
    W_s = nc.alloc_sbuf_tensor("W_s", [VOCAB, VOCAB], BF16).ap()
    RTa_s = nc.alloc_sbuf_tensor("RTa_s", [D + 1, VOCAB], BF16).ap()
    D2b_s = nc.alloc_sbuf_tensor("D2b_s", [5, VOCAB], BF16).ap()
    Zr_s = nc.alloc_sbuf_tensor("Zr_s", [VOCAB, 1], F32).ap()
    Sr_s = nc.alloc_sbuf_tensor("Sr_s", [VOCAB, 1], F32).ap()
    expL_s = nc.alloc_sbuf_tensor("expL_s", [VOCAB, VOCAB], F32).ap()
    Ssum_s = nc.alloc_sbuf_tensor("Ssum_s", [VOCAB, 1], F32).ap()
    Fhi_s = nc.alloc_sbuf_tensor("Fhi_s", [VOCAB, 32], BF16).ap()
    oh_s = nc.alloc_sbuf_tensor("oh_s", [VOCAB, SLICE], BF16).ap()
    outT_s = nc.alloc_sbuf_tensor("outT_s", [128, CHUNK], BF16).ap()
    sq_s = nc.alloc_sbuf_tensor("sq_s", [128, 128], BF16).ap()

    ones128 = const_s[0:128, 0:1]
    ones9 = const_s[0:VOCAB, 0:1]
    A_s = const_s[0:5, 1:5]
    B_s = const_s[0:5, 5:14]
    D2_s = const_s[0:5, 14:23]
    iota9 = const_s[0:VOCAB, 23:24]
    ones128_bf = nc.const_aps.aps[(BF16, 1.0)]
    ones9_bf = ones128_bf[0:VOCAB, 0:1]

    # ================= SP: input DMA, then the gated output DMA =============
    nc.sync.dma_start(x_s, xall).then_inc(sA, 16)

    # ================= ACT: consts DMA + table copies + activations =========
    nc.scalar.dma_start(const_s, consts).then_inc(sC, 16)

    # ================= POOL: xq DMA (SWDGE) + constant memsets ==============
    nc.gpsimd.dma_start(xq_s, xqrep).then_inc(sQ, 16)
    nc.gpsimd.memset(T1_s, 1.0).then_inc(sPL, 1)
    nc.gpsimd.memset(Fhi_s, 0.0).then_inc(sPL, 1)

    # ================= DVE: histogram (9 fused is_equal+accum ops) ==========
    nc.vector.wait_ge(sA, 16)
    for v in range(1, VOCAB):
        nc.vector.tensor_scalar(
            out=ohb[:, v, :],
            in0=x_s,
            scalar1=float(v),
            scalar2=None,
            op0=mybir.AluOpType.is_equal,
            op1=mybir.AluOpType.add,
            accum_out=H[:, v : v + 1],
        ).then_inc(sDVE, 1)

    # ================= ACT: histogram value v=0 while waiting for consts ====
    # relu(1 - x^2) = [x == 0] exactly for integer tokens; the fused accum
    # gives the column sum, taking one op off the DVE histogram's 9
    nc.scalar.wait_ge(sA, 16)
    nc.scalar.activation(
        sq_s, x_s, mybir.ActivationFunctionType.Square
    ).then_inc(sACT, 1)
    nc.scalar.wait_ge(sACT, 1)
    with nc.allow_low_precision(reason="counts <= 128 are exact in bf16"):
        nc.scalar.activation(
            ohb[:, 0, :], sq_s, mybir.ActivationFunctionType.Relu,
            bias=1.0, scale=-1.0, accum_out=H[:, 0:1],
        ).then_inc(sACT, 1)

    # ================= PE: tables (queue order keeps them before c) =========
    nc.tensor.wait_ge(sC, 16)
    nc.tensor.matmul(TT_ps, A_s, B_s).then_inc(sPE, 1)
    nc.tensor.matmul(T_ps, B_s, A_s).then_inc(sPE, 1)

    # ACT: TT_s / T1 copies + E
    nc.scalar.wait_ge(sPE, 1)
    nc.scalar.copy(TT_s, TT_ps).then_inc(sACT, 1)
    nc.scalar.wait_ge(sPE, 2)
    nc.scalar.wait_ge(sPL, 1)
    nc.scalar.copy(T1_s[:, 0:D], T_ps).then_inc(sACT, 1)
    nc.scalar.copy(D2b_s, D2_s).then_inc(sACT, 1)

    nc.tensor.wait_ge(sACT, 3)
    nc.tensor.matmul(G_ps, TT_s, TT_s).then_inc(sPE, 1)
    nc.scalar.wait_ge(sPE, 3)
    nc.scalar.activation(
        E_s, G_ps, mybir.ActivationFunctionType.Exp
    ).then_inc(sACT, 1)

    # c[v] = sum_p H[p, v] — bf16 operands are exact counts, single-pump mm
    nc.tensor.wait_ge(sDVE, 8)
    nc.tensor.wait_ge(sACT, 2)
    nc.tensor.matmul(c_ps, H, ones128_bf).then_inc(sPE, 1)

    # W[v, a] = c_v * exp(G[v, a])
    nc.vector.wait_ge(sPE, 4)
    nc.vector.wait_ge(sACT, 6)
    nc.vector.tensor_scalar(
        out=W_s, in0=E_s, scalar1=c_ps, scalar2=None, op0=mybir.AluOpType.mult
    ).then_inc(sDVE, 1)

    # rows 0-3 = Sh^T, row 4 = Z; Z column for the per-partition exp scale
    nc.tensor.wait_ge(sDVE, 9)
    nc.tensor.matmul(ShTa_ps, T1_s, W_s).then_inc(sPE, 1)
    nc.tensor.matmul(Z_ps, W_s, ones9_bf).then_inc(sPE, 1)

    nc.scalar.wait_ge(sPE, 5)
    nc.scalar.activation(
        RTa_s, ShTa_ps, mybir.ActivationFunctionType.Relu
    ).then_inc(sACT, 1)

    # Zr first (exp's scale must not sit behind the 733ns one-hot), then oh
    nc.vector.wait_ge(sPE, 6)
    nc.vector.reciprocal(Zr_s, Z_ps).then_inc(sDVE, 1)
    nc.vector.wait_ge(sQ, 16)
    nc.vector.tensor_scalar(
        out=oh_s,
        in0=xq_s,
        scalar1=iota9,
        scalar2=None,
        op0=mybir.AluOpType.is_equal,
    ).then_inc(sDVE, 1)

    nc.tensor.wait_ge(sACT, 7)
    nc.tensor.matmul(P_ps, RTa_s, D2b_s).then_inc(sPE, 1)

    # softmax tail: exp on ACT, sum/recip/scale on DVE
    nc.scalar.wait_ge(sPE, 7)
    nc.scalar.wait_ge(sDVE, 10)
    nc.scalar.activation(
        expL_s, P_ps, mybir.ActivationFunctionType.Exp, scale=Zr_s
    ).then_inc(sACT, 1)
    nc.vector.wait_ge(sACT, 8)
    nc.vector.tensor_reduce(
        Ssum_s, expL_s, axis=mybir.AxisListType.X, op=mybir.AluOpType.add
    ).then_inc(sDVE, 1)
    # the reduce writes via the accumulator path: even same-engine consumers
    # need a semaphore on its completion
    nc.vector.wait_ge(sDVE, 12)
    nc.vector.reciprocal(Sr_s, Ssum_s).then_inc(sDVE, 1)
    nc.vector.wait_ge(sDVE, 13)
    nc.vector.wait_ge(sPL, 2)
    nc.vector.tensor_scalar(
        out=Fhi_s[:, 0:VOCAB],
        in0=expL_s,
        scalar1=Sr_s,
        scalar2=None,
        op0=mybir.AluOpType.mult,
    ).then_inc(sDVE, 1)

    # gather: four concurrent 32-col strips into one psum bank
    nc.tensor.wait_ge(sDVE, 14)
    for cidx in range(NCHUNK):
        sl = slice(cidx * CHUNK, (cidx + 1) * CHUNK)
        nc.tensor.matmul(
            o_ps[32 * cidx : 32 * cidx + 32, :],
            Fhi_s,
            oh_s[:, sl],
            start=True,
            stop=True,
            tile_position=(0, 32 * cidx),
            skip_group_check=True,
        ).then_inc(sPE, 1)

    # eviction: one ACT copy f32 -> bf16 (ACT+DVE split halves hit the
    # cayman event-accel cross-engine deadlock on HW — tile serializes the
    # pair for the same reason — so a single copy is both safe and as fast)
    nc.scalar.wait_ge(sPE, 11)
    nc.scalar.copy(outT_s, o_ps).then_inc(sACT, 1)

    # output DMA on SP, gated on the evict. No engine waits on the completion
    # semaphore: the NEFF epilogue's per-engine semaphore-clear phase (~6us,
    # started at the post-kernel rendezvous) strictly covers the remaining
    # DMA flight time, so the output always lands well before the NEFF can
    # signal completion.
    nc.sync.wait_ge(sACT, 9)
    nc.sync.dma_start(outT, outT_s).then_inc(sO, 16)


def host_prep(x, emb, proj_w, proj_b, forw_w, forw_b, prj_w, prj_b):
    """Pack weights/constants and per-core sharded inputs."""
    f32 = np.float32
    x = np.asarray(x).reshape(-1).astype(np.int64)
    assert x.shape == (S,)
    emb = np.asarray(emb, f32)
    proj_w = np.asarray(proj_w, f32)
    proj_b = np.asarray(proj_b, f32)
    forw_w = np.asarray(forw_w, f32)
    forw_b = np.asarray(forw_b, f32)
    prj_w = np.asarray(prj_w, f32)
    prj_b = np.asarray(prj_b, f32)

    M2 = (prj_w @ forw_w).astype(f32)          # (9, 4)
    b2 = (prj_w @ forw_b + prj_b).astype(f32)  # (9,)

    consts = np.zeros((128, NCONST), f32)
    consts[:, 0] = 1.0
    consts[0:4, 1:5] = proj_w.T
    consts[4, 1:5] = proj_b
    consts[0:4, 5:14] = emb.T
    consts[4, 5:14] = 1.0
    consts[0:4, 14:23] = M2.T
    consts[4, 14:23] = b2
    consts[0:VOCAB, 23] = np.arange(VOCAB, dtype=f32)

    xall = x.reshape(128, 128).astype(np.uint8)
    in_maps = []
    for i in range(NCORES):
        xq = x[i * SLICE : (i + 1) * SLICE].astype(ml_dtypes.bfloat16)
        in_maps.append(
            {
                "xall": xall,
                "consts": consts,
                "xqrep": np.ascontiguousarray(
                    np.broadcast_to(xq[None, :], (VOCAB, SLICE))
                ),
            }
        )
    return in_maps


def unpack_out(arr):
    """outT [128, CHUNK] bf16 -> (SLICE, VOCAB) f32 for one core."""
    a = np.asarray(arr).astype(np.float32)
    return a.reshape(NCHUNK, 32, CHUNK)[:, :VOCAB, :].transpose(0, 2, 1).reshape(
        SLICE, VOCAB
    )


_NC_CACHE = None


def kernel(x, emb, proj_w, proj_b, forw_w, forw_b, prj_w, prj_b):
    global _NC_CACHE, LAST_RESULTS
    if _NC_CACHE is None:
        _NC_CACHE = build_nc()
    nc = _NC_CACHE
    in_maps = host_prep(x, emb, proj_w, proj_b, forw_w, forw_b, prj_w, prj_b)
    trace = bool(os.environ.get("BASS_TRACE"))
    res = run_bass_kernel_spmd(nc, in_maps, list(range(NCORES)), trace=trace)
    LAST_RESULTS = res
    out = np.empty((S, VOCAB), np.float32)
    for i in range(NCORES):
        out[i * SLICE : (i + 1) * SLICE, :] = unpack_out(res.results[i]["outT"])
    return out
